# revision 44
# baseline (speedup 1.0000x reference)
"""Single-head attention (B=8, S=2048, D=384) on 8 NeuronCores.

Sharding: data-parallel over batch — core b computes batch element b
entirely (the module is single-headed with no cross-batch coupling), all
three weight matrices replicated. Host marshalling is layout-only (zero
FLOPs): x pre-transposed/tiled to [128, 3, S], weights pre-tiled to
[128, 3, 384] (Wq/Wk natural [e,d]; Wv transposed [d,e]).

Per-core dataflow (f32 in/out, matmuls in float32r at full PE rate):
  - QK fold: scores = (x Wq^T)(x Wk^T)^T = x (Wq^T Wk) x^T. M = Wq^T Wk
    costs 9 tiny matmuls and replaces the separate Q and K projections
    with ONE projection TT = M^T x^T, saving ~15k PE cycles/core.
  - V = x Wv^T in natural [S, D] layout with two ones-columns appended.
  - alphaT[k, q] tiles = xT^T @ TT accumulated over 3 d-tiles; exp on
    ScalarE (|logit| < 60 so fp32 exp cannot overflow; softmax is
    shift-invariant so no max subtraction is needed).
  - PV and the softmax denominator accumulate together in one PSUM tile
    via the ones-columns (column D = sum_k exp); out = raw * recip(den).

PE warm-up: the HAM clock gate holds the PE at 1.2 GHz until ~3.4us of
sustained activity and re-cools after idle. Dependency-free fp32 dummy
matmuls start ~7.3us (before any DMA lands) and f32r pads bridge until
the first inputs arrive (~16.5us), so every real matmul runs at 2.4 GHz
and the PE stream then runs gap-free to the end.

DMA: weights ride the sync HWDGE queue (FIFO, fast first bytes); x
chunks ride the gpsimd software DGE in consumption order — slower to
start, but paced exactly like the projection loop consumes them, and
the two DGE subsystems stream in parallel.
"""

import os
import numpy as np

import concourse.bacc as bacc
import concourse.tile as tile
from concourse import mybir
from concourse import bass_utils

P = 128
S = 2048
D = 384
NB = 8
DT = D // P
ST = S // P
QC = 512
NQ = S // QC
F32 = mybir.dt.float32
F32R = mybir.dt.float32r
BF16 = mybir.dt.bfloat16

MM_MODE = os.environ.get("ATT_MM_MODE", "f32r")
N_WARM_F32 = int(os.environ.get("ATT_WARM_F32", "4"))
N_WARM = int(os.environ.get("ATT_WARM", "18"))


def _build():
    sb_dt = F32R
    pv_dt = BF16 if MM_MODE == "hybrid" else F32R

    nc = bacc.Bacc(
        "TRN2", target_bir_lowering=False, debug=False, enable_asserts=False
    )
    xt = nc.dram_tensor("xt", [P, DT, S], F32R, kind="ExternalInput").ap()
    wqn = nc.dram_tensor("wqn", [P, DT, D], F32R, kind="ExternalInput").ap()
    wkn = nc.dram_tensor("wkn", [P, DT, D], F32R, kind="ExternalInput").ap()
    wvt = nc.dram_tensor("wvt", [P, DT, D], F32R, kind="ExternalInput").ap()
    out = nc.dram_tensor("out", [S, D], F32, kind="ExternalOutput").ap()

    with tile.TileContext(nc) as tc:
        with (
            tc.tile_pool(name="const", bufs=1) as const_pool,
            tc.tile_pool(name="big", bufs=1) as big,
            tc.tile_pool(name="expool", bufs=4) as ex_pool,
            tc.tile_pool(name="obpool", bufs=3) as ob_pool,
            tc.tile_pool(name="smalls", bufs=4) as small_pool,
            tc.tile_pool(name="ps_stage", bufs=4, space="PSUM") as ps_stage,
            tc.tile_pool(name="ps_acc", bufs=4, space="PSUM") as ps_acc,
        ):
            # constants on gpsimd: it exits the framework preamble first
            # (~5.9us), so the fp32 prewarm matmuls can start earliest
            ones_c = const_pool.tile([P, 2], F32, tag="ones", name="ones_c")
            warm_z = const_pool.tile([P, QC], F32, tag="warmz", name="warm_z")
            nc.gpsimd.memset(ones_c, 1.0)
            nc.gpsimd.memset(warm_z, 0.0)
            warm_w = const_pool.tile([P, P], sb_dt, tag="warmw", name="warm_w")
            warm_m = const_pool.tile([P, QC], sb_dt, tag="warmm", name="warm_m")
            nc.vector.tensor_copy(warm_m, warm_z)
            nc.vector.tensor_copy(warm_w, warm_z[:, 0:P])

            xT = big.tile([P, DT, S], sb_dt, tag="xT", name="xT")
            tT = big.tile([P, DT, S], sb_dt, tag="tT", name="tT")
            vA = big.tile([P, ST, D + 2], pv_dt, tag="vA", name="vA")
            wqT = big.tile([P, DT, D], sb_dt, tag="wqT", name="wqT")
            wkT = big.tile([P, DT, D], sb_dt, tag="wkT", name="wkT")
            wvT = big.tile([P, DT, D], sb_dt, tag="wvT", name="wvT")
            mT = big.tile([P, DT, D], sb_dt, tag="mT", name="mT")

            _proj_n = [0]

            def proj_tile():
                _proj_n[0] += 1
                if _proj_n[0] % 2:
                    return ps_stage.tile([P, QC], F32, tag="ps1", name="pj")
                return ps_acc.tile([P, QC], F32, tag="acc", name="pj")

            # PE prewarm + keep-warm bridge: the HAM clock gate needs ~3.4us
            # of sustained PE activity to unthrottle 1.2 -> 2.4 GHz, and any
            # multi-us idle re-cools it. fp32 dummies (ones x zeros, no DVE
            # dependency) start ~7.3us; the f32r pads bridge until the first
            # input DMAs complete (~16.5us) so all real work runs at 2.4 GHz.
            for _ in range(N_WARM_F32):
                pw = proj_tile()
                nc.tensor.matmul(
                    pw[0:2, :], ones_c, warm_z, start=True, stop=True
                )
            for _ in range(N_WARM):
                pw = proj_tile()
                nc.tensor.matmul(pw, warm_w, warm_m, start=True, stop=True)

            nc.sync.dma_start(out=wvT, in_=wvt)
            nc.sync.dma_start(out=wqT, in_=wqn)
            nc.sync.dma_start(out=wkT, in_=wkn)
            for lo, hi in ((0, QC // 2), (QC // 2, QC)):
                nc.gpsimd.dma_start(out=xT[:, :, lo:hi], in_=xt[:, :, lo:hi])
            for qc in range(1, NQ):
                nc.gpsimd.dma_start(
                    out=xT[:, :, qc * QC:(qc + 1) * QC],
                    in_=xt[:, :, qc * QC:(qc + 1) * QC],
                )

            def project_v(st):
                pv = proj_tile()
                for dt_ in range(DT):
                    nc.tensor.matmul(
                        pv[:, 0:D],
                        xT[:, dt_, st * P:(st + 1) * P],
                        wvT[:, dt_, :],
                        start=(dt_ == 0),
                        stop=(dt_ == DT - 1),
                    )
                nc.vector.tensor_copy(vA[:, st, 0:D], pv[:, 0:D])

            def compute_m():
                for dt_ in range(DT):
                    pm = proj_tile()
                    for et in range(DT):
                        nc.tensor.matmul(
                            pm[:, 0:D],
                            wqT[:, et, dt_ * P:(dt_ + 1) * P],
                            wkT[:, et, :],
                            start=(et == 0),
                            stop=(et == DT - 1),
                        )
                    nc.vector.tensor_copy(mT[:, dt_, :], pm[:, 0:D])

            def project_t_chunk(qc, et):
                pp = proj_tile()
                for dt_ in range(DT):
                    nc.tensor.matmul(
                        pp,
                        mT[:, dt_, et * P:(et + 1) * P],
                        xT[:, dt_, qc * QC:(qc + 1) * QC],
                        start=(dt_ == 0),
                        stop=(dt_ == DT - 1),
                    )
                nc.vector.tensor_copy(tT[:, et, qc * QC:(qc + 1) * QC], pp)

            for st in range(4):
                project_v(st)
            compute_m()
            for et in range(DT):
                project_t_chunk(0, et)
            for qc in range(1, NQ):
                for st in range(qc * 4, qc * 4 + 4):
                    project_v(st)
                for et in range(DT):
                    project_t_chunk(qc, et)
            nc.vector.tensor_copy(
                vA[:, :, D:D + 2],
                ones_c.unsqueeze(1).broadcast_to([P, ST, 2]),
            )

            for c in range(NQ):
                accs = [
                    ps_acc.tile([P, D + 2], F32, tag="acc", name="acc")
                    for _ in range(4)
                ]

                def emit_pv(kt_i, ex):
                    for qs in range(4):
                        nc.tensor.matmul(
                            accs[qs],
                            ex[:, qs * P:(qs + 1) * P],
                            vA[:, kt_i, :],
                            start=(kt_i == 0),
                            stop=(kt_i == ST - 1),
                        )

                pending = []
                for kt_i in range(ST):
                    pa = ps_stage.tile([P, QC], F32, tag="ps1", name="pa")
                    for et in range(DT):
                        nc.tensor.matmul(
                            pa,
                            xT[:, et, kt_i * P:(kt_i + 1) * P],
                            tT[:, et, c * QC:(c + 1) * QC],
                            start=(et == 0),
                            stop=(et == DT - 1),
                        )
                    ex = ex_pool.tile([P, QC], pv_dt, tag="ex", name="ex")
                    nc.scalar.activation(
                        ex, pa, mybir.ActivationFunctionType.Exp
                    )
                    pending.append((kt_i, ex))
                    if len(pending) > 2:
                        emit_pv(*pending.pop(0))
                # flush qs-OUTER: acc[0] finishes ~1.5us before acc[3], so
                # the chunk epilogue overlaps the remaining PV matmuls
                for qs in range(4):
                    for kt_i, ex in pending:
                        nc.tensor.matmul(
                            accs[qs],
                            ex[:, qs * P:(qs + 1) * P],
                            vA[:, kt_i, :],
                            start=False,
                            stop=(kt_i == ST - 1),
                        )

                recs = []
                for qs in range(4):
                    rec = small_pool.tile([P, 1], F32, tag="rec", name="rec")
                    nc.vector.reciprocal(rec, accs[qs][:, D:D + 1])
                    recs.append(rec)
                for qs in range(4):
                    ob = ob_pool.tile([P, D], F32, tag="ob", name="ob")
                    qt_row = (c * 4 + qs) * P
                    if qs % 2:
                        nc.scalar.activation(
                            ob,
                            accs[qs][:, 0:D],
                            mybir.ActivationFunctionType.Copy,
                            scale=recs[qs],
                        )
                        nc.scalar.dma_start(
                            out=out[qt_row:qt_row + P, :], in_=ob
                        )
                    else:
                        nc.vector.tensor_scalar_mul(
                            ob, accs[qs][:, 0:D], recs[qs]
                        )
                        nc.sync.dma_start(
                            out=out[qt_row:qt_row + P, :], in_=ob
                        )

    nc.compile()
    return nc


_NC = None
_FAST = None


def _get_nc():
    global _NC
    if _NC is None:
        _NC = _build()
    return _NC


def _fast_runner():
    global _FAST
    if _FAST is not None:
        return _FAST
    import jax
    from jax.experimental.shard_map import shard_map
    from jax.sharding import Mesh, PartitionSpec

    from concourse import bass2jax

    nc = _get_nc()
    bass2jax.install_neuronx_cc_hook()

    in_names = ["xt", "wqn", "wkn", "wvt"]
    out_aval = jax.core.ShapedArray((S, D), np.float32)

    def _body(*args):
        operands = list(args)
        operands.append(bass2jax.partition_id_tensor())
        outs = bass2jax._bass_exec_p.bind(
            *operands,
            out_avals=(out_aval,),
            in_names=tuple(in_names) + ("out", "partition_id"),
            out_names=("out",),
            lowering_input_output_aliases=(),
            sim_require_finite=True,
            sim_require_nnan=True,
            nc=nc,
        )
        return tuple(outs)

    devices = jax.devices()[:NB]
    mesh = Mesh(np.asarray(devices), ("core",))
    n_in = len(in_names) + 1
    fn = jax.jit(
        shard_map(
            _body,
            mesh=mesh,
            in_specs=(PartitionSpec("core"),) * n_in,
            out_specs=(PartitionSpec("core"),),
            check_rep=False,
        ),
        donate_argnums=(n_in - 1,),
        keep_unused=True,
    )
    _FAST = fn
    return fn


def _tile_ed(w):
    return np.ascontiguousarray(
        w.reshape(DT, P, w.shape[1]).transpose(1, 0, 2)
    )


def _marshal(att_input, Wq, Wk, Wv):
    att_input = np.asarray(att_input, dtype=np.float32)
    xts = np.ascontiguousarray(
        att_input.transpose(0, 2, 1)
        .reshape(NB, DT, P, S)
        .transpose(0, 2, 1, 3)
    )
    wq = _tile_ed(np.asarray(Wq, dtype=np.float32))
    wk = _tile_ed(np.asarray(Wk, dtype=np.float32))
    wv = _tile_ed(np.ascontiguousarray(np.asarray(Wv, np.float32).T))
    return xts, (wq, wk, wv)


def run(att_input, Wq, Wk, Wv, trace=False):
    xts, wts = _marshal(att_input, Wq, Wk, Wv)
    if trace:
        in_maps = [
            {"xt": xts[b], "wqn": wts[0], "wkn": wts[1], "wvt": wts[2]}
            for b in range(NB)
        ]
        res = bass_utils.run_bass_kernel_spmd(
            _get_nc(), in_maps, core_ids=list(range(NB)), trace=True
        )
        out = np.stack([res.results[b]["out"] for b in range(NB)], axis=0)
        return out.astype(np.float32, copy=False), res

    try:
        fn = _fast_runner()
        xs = xts.reshape(NB * P, DT, S)
        ws = [
            np.concatenate([w] * NB, axis=0).reshape(NB * P, DT, D)
            for w in wts
        ]
        zeros = np.zeros((NB * S, D), np.float32)
        (out,) = fn(xs, *ws, zeros)
        out = np.asarray(out)
    except Exception:
        in_maps = [
            {"xt": xts[b], "wqn": wts[0], "wkn": wts[1], "wvt": wts[2]}
            for b in range(NB)
        ]
        res = bass_utils.run_bass_kernel_spmd(
            _get_nc(), in_maps, core_ids=list(range(NB))
        )
        out = np.stack([res.results[b]["out"] for b in range(NB)], axis=0)
    return out.reshape(NB, S, D).astype(np.float32, copy=False), None


def kernel(att_input, Wq, Wk, Wv):
    out, _ = run(att_input, Wq, Wk, Wv)
    return out


# revision 45
# speedup vs baseline: 1.0098x; 1.0098x over previous
"""Single-head attention (B=8, S=2048, D=384) on 8 NeuronCores.

Sharding: data-parallel over batch — core b computes batch element b
entirely (the module is single-headed with no cross-batch coupling), all
three weight matrices replicated. Host marshalling is layout-only (zero
FLOPs): x pre-transposed/tiled to [128, 3, S], weights pre-tiled to
[128, 3, 384] (Wq/Wk natural [e,d]; Wv transposed [d,e]).

Per-core dataflow (f32 in/out, matmuls in float32r at full PE rate):
  - QK fold: scores = (x Wq^T)(x Wk^T)^T = x (Wq^T Wk) x^T. M = Wq^T Wk
    costs 9 tiny matmuls and replaces the separate Q and K projections
    with ONE projection TT = M^T x^T, saving ~15k PE cycles/core.
  - V = x Wv^T in natural [S, D] layout with two ones-columns appended.
  - alphaT[k, q] tiles = xT^T @ TT accumulated over 3 d-tiles; exp on
    ScalarE (|logit| < 60 so fp32 exp cannot overflow; softmax is
    shift-invariant so no max subtraction is needed).
  - PV and the softmax denominator accumulate together in one PSUM tile
    via the ones-columns (column D = sum_k exp); out = raw * recip(den).

PE warm-up: the HAM clock gate holds the PE at 1.2 GHz until ~3.4us of
sustained activity and re-cools after idle. Dependency-free fp32 dummy
matmuls start ~7.3us (before any DMA lands) and f32r pads bridge until
the first inputs arrive (~16.5us), so every real matmul runs at 2.4 GHz
and the PE stream then runs gap-free to the end.

DMA: weights ride the sync HWDGE queue (FIFO, fast first bytes); x
chunks ride the gpsimd software DGE in consumption order — slower to
start, but paced exactly like the projection loop consumes them, and
the two DGE subsystems stream in parallel.
"""

import os
import numpy as np

import concourse.bacc as bacc
import concourse.tile as tile
from concourse import mybir
from concourse import bass_utils

P = 128
S = 2048
D = 384
NB = 8
DT = D // P
ST = S // P
QC = 512
NQ = S // QC
F32 = mybir.dt.float32
F32R = mybir.dt.float32r
BF16 = mybir.dt.bfloat16

MM_MODE = os.environ.get("ATT_MM_MODE", "f32r")
N_WARM_F32 = int(os.environ.get("ATT_WARM_F32", "4"))
N_WARM = int(os.environ.get("ATT_WARM", "18"))


def _build():
    sb_dt = F32R
    pv_dt = BF16 if MM_MODE == "hybrid" else F32R

    nc = bacc.Bacc(
        "TRN2", target_bir_lowering=False, debug=False, enable_asserts=False
    )
    xt = nc.dram_tensor("xt", [P, DT, S], F32R, kind="ExternalInput").ap()
    wqn = nc.dram_tensor("wqn", [P, DT, D], F32R, kind="ExternalInput").ap()
    wkn = nc.dram_tensor("wkn", [P, DT, D], F32R, kind="ExternalInput").ap()
    wvt = nc.dram_tensor("wvt", [P, DT, D], F32R, kind="ExternalInput").ap()
    out = nc.dram_tensor("out", [S, D], F32, kind="ExternalOutput").ap()

    with tile.TileContext(nc) as tc:
        with (
            tc.tile_pool(name="const", bufs=1) as const_pool,
            tc.tile_pool(name="big", bufs=1) as big,
            tc.tile_pool(name="expool", bufs=4) as ex_pool,
            tc.tile_pool(name="obpool", bufs=3) as ob_pool,
            tc.tile_pool(name="smalls", bufs=4) as small_pool,
            tc.tile_pool(name="ps_stage", bufs=4, space="PSUM") as ps_stage,
            tc.tile_pool(name="ps_acc", bufs=4, space="PSUM") as ps_acc,
        ):
            # constants on gpsimd: it exits the framework preamble first
            # (~5.9us), so the fp32 prewarm matmuls can start earliest
            ones_c = const_pool.tile([P, 2], F32, tag="ones", name="ones_c")
            warm_z = const_pool.tile([P, QC], F32, tag="warmz", name="warm_z")
            nc.gpsimd.memset(ones_c, 1.0)
            nc.gpsimd.memset(warm_z, 0.0)
            warm_w = const_pool.tile([P, P], sb_dt, tag="warmw", name="warm_w")
            warm_m = const_pool.tile([P, QC], sb_dt, tag="warmm", name="warm_m")
            nc.vector.tensor_copy(warm_m, warm_z)
            nc.vector.tensor_copy(warm_w, warm_z[:, 0:P])

            xT = big.tile([P, DT, S], sb_dt, tag="xT", name="xT")
            tT = big.tile([P, DT, S], sb_dt, tag="tT", name="tT")
            vA = big.tile([P, ST, D + 2], pv_dt, tag="vA", name="vA")
            wqT = big.tile([P, DT, D], sb_dt, tag="wqT", name="wqT")
            wkT = big.tile([P, DT, D], sb_dt, tag="wkT", name="wkT")
            wvT = big.tile([P, DT, D], sb_dt, tag="wvT", name="wvT")
            mT = big.tile([P, DT, D], sb_dt, tag="mT", name="mT")

            _proj_n = [0]

            def proj_tile():
                _proj_n[0] += 1
                if _proj_n[0] % 2:
                    return ps_stage.tile([P, QC], F32, tag="ps1", name="pj")
                return ps_acc.tile([P, QC], F32, tag="acc", name="pj")

            # PE prewarm + keep-warm bridge: the HAM clock gate needs ~3.4us
            # of sustained PE activity to unthrottle 1.2 -> 2.4 GHz, and any
            # multi-us idle re-cools it. fp32 dummies (ones x zeros, no DVE
            # dependency) start ~7.3us; the f32r pads bridge until the first
            # input DMAs complete (~16.5us) so all real work runs at 2.4 GHz.
            for _ in range(N_WARM_F32):
                pw = proj_tile()
                nc.tensor.matmul(
                    pw[0:2, :], ones_c, warm_z, start=True, stop=True
                )
            for _ in range(N_WARM):
                pw = proj_tile()
                nc.tensor.matmul(pw, warm_w, warm_m, start=True, stop=True)

            nc.sync.dma_start(out=wvT, in_=wvt)
            nc.sync.dma_start(out=wqT, in_=wqn)
            nc.sync.dma_start(out=wkT, in_=wkn)
            for lo, hi in ((0, QC // 2), (QC // 2, QC)):
                nc.gpsimd.dma_start(out=xT[:, :, lo:hi], in_=xt[:, :, lo:hi])
            for qc in range(1, NQ):
                nc.gpsimd.dma_start(
                    out=xT[:, :, qc * QC:(qc + 1) * QC],
                    in_=xt[:, :, qc * QC:(qc + 1) * QC],
                )

            def project_v(st):
                pv = proj_tile()
                for dt_ in range(DT):
                    nc.tensor.matmul(
                        pv[:, 0:D],
                        xT[:, dt_, st * P:(st + 1) * P],
                        wvT[:, dt_, :],
                        start=(dt_ == 0),
                        stop=(dt_ == DT - 1),
                    )
                nc.vector.tensor_copy(vA[:, st, 0:D], pv[:, 0:D])

            def compute_m():
                for dt_ in range(DT):
                    pm = proj_tile()
                    for et in range(DT):
                        nc.tensor.matmul(
                            pm[:, 0:D],
                            wqT[:, et, dt_ * P:(dt_ + 1) * P],
                            wkT[:, et, :],
                            start=(et == 0),
                            stop=(et == DT - 1),
                        )
                    nc.vector.tensor_copy(mT[:, dt_, :], pm[:, 0:D])

            def project_t_chunk(qc, et):
                pp = proj_tile()
                for dt_ in range(DT):
                    nc.tensor.matmul(
                        pp,
                        mT[:, dt_, et * P:(et + 1) * P],
                        xT[:, dt_, qc * QC:(qc + 1) * QC],
                        start=(dt_ == 0),
                        stop=(dt_ == DT - 1),
                    )
                nc.vector.tensor_copy(tT[:, et, qc * QC:(qc + 1) * QC], pp)

            for st in range(4):
                project_v(st)
            compute_m()
            for et in range(DT):
                project_t_chunk(0, et)
            for qc in range(1, NQ):
                for st in range(qc * 4, qc * 4 + 4):
                    project_v(st)
                for et in range(DT):
                    project_t_chunk(qc, et)
            nc.vector.tensor_copy(
                vA[:, :, D:D + 2],
                ones_c.unsqueeze(1).broadcast_to([P, ST, 2]),
            )

            for c in range(NQ):
                accs = [
                    ps_acc.tile([P, D + 2], F32, tag="acc", name="acc")
                    for _ in range(4)
                ]

                def emit_pv(kt_i, ex):
                    for qs in range(4):
                        nc.tensor.matmul(
                            accs[qs],
                            ex[:, qs * P:(qs + 1) * P],
                            vA[:, kt_i, :],
                            start=(kt_i == 0),
                            stop=(kt_i == ST - 1),
                        )

                pending = []
                for kt_i in range(ST):
                    pa = ps_stage.tile([P, QC], F32, tag="ps1", name="pa")
                    for et in range(DT):
                        nc.tensor.matmul(
                            pa,
                            xT[:, et, kt_i * P:(kt_i + 1) * P],
                            tT[:, et, c * QC:(c + 1) * QC],
                            start=(et == 0),
                            stop=(et == DT - 1),
                        )
                    ex = ex_pool.tile([P, QC], pv_dt, tag="ex", name="ex")
                    nc.scalar.activation(
                        ex, pa, mybir.ActivationFunctionType.Exp
                    )
                    pending.append((kt_i, ex))
                    if len(pending) > 2:
                        emit_pv(*pending.pop(0))
                for item in pending:
                    emit_pv(*item)

                recs = []
                for qs in range(4):
                    rec = small_pool.tile([P, 1], F32, tag="rec", name="rec")
                    nc.vector.reciprocal(rec, accs[qs][:, D:D + 1])
                    recs.append(rec)
                for qs in range(4):
                    ob = ob_pool.tile([P, D], F32, tag="ob", name="ob")
                    qt_row = (c * 4 + qs) * P
                    if qs % 2:
                        nc.scalar.activation(
                            ob,
                            accs[qs][:, 0:D],
                            mybir.ActivationFunctionType.Copy,
                            scale=recs[qs],
                        )
                        nc.scalar.dma_start(
                            out=out[qt_row:qt_row + P, :], in_=ob
                        )
                    else:
                        nc.vector.tensor_scalar_mul(
                            ob, accs[qs][:, 0:D], recs[qs]
                        )
                        nc.sync.dma_start(
                            out=out[qt_row:qt_row + P, :], in_=ob
                        )

    nc.compile()
    return nc


_NC = None
_FAST = None


def _get_nc():
    global _NC
    if _NC is None:
        _NC = _build()
    return _NC


def _fast_runner():
    global _FAST
    if _FAST is not None:
        return _FAST
    import jax
    from jax.experimental.shard_map import shard_map
    from jax.sharding import Mesh, PartitionSpec

    from concourse import bass2jax

    nc = _get_nc()
    bass2jax.install_neuronx_cc_hook()

    in_names = ["xt", "wqn", "wkn", "wvt"]
    out_aval = jax.core.ShapedArray((S, D), np.float32)

    def _body(*args):
        operands = list(args)
        operands.append(bass2jax.partition_id_tensor())
        outs = bass2jax._bass_exec_p.bind(
            *operands,
            out_avals=(out_aval,),
            in_names=tuple(in_names) + ("out", "partition_id"),
            out_names=("out",),
            lowering_input_output_aliases=(),
            sim_require_finite=True,
            sim_require_nnan=True,
            nc=nc,
        )
        return tuple(outs)

    devices = jax.devices()[:NB]
    mesh = Mesh(np.asarray(devices), ("core",))
    n_in = len(in_names) + 1
    fn = jax.jit(
        shard_map(
            _body,
            mesh=mesh,
            in_specs=(PartitionSpec("core"),) * n_in,
            out_specs=(PartitionSpec("core"),),
            check_rep=False,
        ),
        donate_argnums=(n_in - 1,),
        keep_unused=True,
    )
    _FAST = fn
    return fn


def _tile_ed(w):
    return np.ascontiguousarray(
        w.reshape(DT, P, w.shape[1]).transpose(1, 0, 2)
    )


def _marshal(att_input, Wq, Wk, Wv):
    att_input = np.asarray(att_input, dtype=np.float32)
    xts = np.ascontiguousarray(
        att_input.transpose(0, 2, 1)
        .reshape(NB, DT, P, S)
        .transpose(0, 2, 1, 3)
    )
    wq = _tile_ed(np.asarray(Wq, dtype=np.float32))
    wk = _tile_ed(np.asarray(Wk, dtype=np.float32))
    wv = _tile_ed(np.ascontiguousarray(np.asarray(Wv, np.float32).T))
    return xts, (wq, wk, wv)


def run(att_input, Wq, Wk, Wv, trace=False):
    xts, wts = _marshal(att_input, Wq, Wk, Wv)
    if trace:
        in_maps = [
            {"xt": xts[b], "wqn": wts[0], "wkn": wts[1], "wvt": wts[2]}
            for b in range(NB)
        ]
        res = bass_utils.run_bass_kernel_spmd(
            _get_nc(), in_maps, core_ids=list(range(NB)), trace=True
        )
        out = np.stack([res.results[b]["out"] for b in range(NB)], axis=0)
        return out.astype(np.float32, copy=False), res

    try:
        fn = _fast_runner()
        xs = xts.reshape(NB * P, DT, S)
        ws = [
            np.concatenate([w] * NB, axis=0).reshape(NB * P, DT, D)
            for w in wts
        ]
        zeros = np.zeros((NB * S, D), np.float32)
        (out,) = fn(xs, *ws, zeros)
        out = np.asarray(out)
    except Exception:
        in_maps = [
            {"xt": xts[b], "wqn": wts[0], "wkn": wts[1], "wvt": wts[2]}
            for b in range(NB)
        ]
        res = bass_utils.run_bass_kernel_spmd(
            _get_nc(), in_maps, core_ids=list(range(NB))
        )
        out = np.stack([res.results[b]["out"] for b in range(NB)], axis=0)
    return out.reshape(NB, S, D).astype(np.float32, copy=False), None


def kernel(att_input, Wq, Wk, Wv):
    out, _ = run(att_input, Wq, Wk, Wv)
    return out
